# revision 12
# baseline (speedup 1.0000x reference)
"""Trainium2 Bass kernel for nn_ModelStep_12120397709837.

Integrates the 6-state SBML pharmacokinetic ODE for B=524288 trajectories over
deltaT, matching jax.experimental.ode.odeint(rtol=atol=1e-6) semantics.

Method (per trajectory, fixed op sequence, no data-dependent control flow):
  - y1 is exact exponential decay (one constant multiply).
  - y3 + y5 = S is exactly conserved; y5 is eliminated.
  - y0 and y4 follow closed-form linear decays; their tiny x-coupling is
    applied once at the end via a trapezoid correction.
  - Strang splitting with N_STEPS uniform steps on (u=y2, x=y3):
      * slow flow: u <- M22*u + M23*x + alpha_j*y0(0) + beta_j*y4(0), with all
        coefficients precomputed on host from the 4x4 matrix exponential;
      * fast flow: reversible bimolecular binding (u + v <-> x), solved
        EXACTLY via the closed-form Riccati solution:
          xi(h) = 2*C*(1-E) / (bp - bm*E),  E = exp(-h*K*sd)
          C = u*v-(d/K)*x, B = u+v+d/K, sd = sqrt(B^2-4C), bm/bp = B-/+sd
  sqrt and reciprocal are computed as Exp(0.5*Ln(.)) / Exp(-Ln(.)) so all ACT
  functions stay in one table set (natural_log_exp_and_others) - no reloads.

Sharding: data-parallel over the batch axis, 65536 trajectories per core on
8 NeuronCores, laid out [128 partitions x 512].
"""
import sys
sys.path.insert(0, '/opt/trn_rl_repo')

import numpy as np

N_CORES = 8
B_TOTAL = 524288
B_CORE = B_TOTAL // N_CORES          # 65536
P = 128                              # SBUF partitions
JTOT = B_CORE // P                   # 512 trajectories per partition
NBLK = 1                             # column blocks
JBLK = JTOT // NBLK
N_STEPS = 8

_cache = {}


def _derive(c64, T, n):
    """Host-side f64 derivation of all scalar constants baked into the program."""
    from scipy.linalg import expm
    c = c64
    a   = (c[6]-c[8] + (1.0-c[14])*c[8] + c[12]) / (c[24]*1000.0)
    b   = c[12]*c[13] / (c[24]*5.0)
    g   = c[7] / (c[25]*55.0)
    c20 = c[12] / (c[27]*1000.0)
    c24 = c[12] / (c[27]*45.0)
    c23 = c[10] / 5.0
    c22 = c[11] / 5.0
    K   = c[9] / 25.0
    d   = c[10]/5.0 + c[12]/(c[27]*5.0)
    e40 = (1.0-c[14])*c[8] / (c[26]*1000.0)
    e43 = c[12]*(1.0-c[13]) / (c[26]*5.0)
    e44 = ((1.0-c[15])*c[8]/45.0 + c[12]/45.0) / c[26]
    h = T / n
    A = np.array([
        [-a,    0.0,  0.0,  b],
        [e40, -e44,   0.0,  e43],
        [c20,  c24, -c22,  (c23 - d)],
        [0.0,  0.0,  0.0,  0.0]])

    def psi(t):  # y0(0) -> y4(t) double-exponential response
        return e40*(np.exp(-a*t)-np.exp(-e44*t))/(e44-a)

    # slow application j: j=0 half step (before first fast), j=1..n-1 full,
    # j=n half (after last fast). Tj = cumulative slow time before app j.
    alphas, betas, m22s, m23s = [], [], [], []
    for j in range(n+1):
        hh = h/2 if j in (0, n) else h
        Tj = 0.0 if j == 0 else h/2 + (j-1)*h
        M = expm(A*hh)
        alphas.append(M[2, 0]*np.exp(-a*Tj) + M[2, 1]*psi(Tj))
        betas.append(M[2, 1]*np.exp(-e44*Tj))
        m22s.append(M[2, 2])
        m23s.append(M[2, 3])
    return dict(alphas=alphas, betas=betas, m22s=m22s, m23s=m23s,
                dK=d/K, hK=h*K,
                Dy0T=np.exp(-a*T), Dy4T=np.exp(-e44*T), PsiT=psi(T),
                cx_y0=b*T*np.exp(-a*T/2.0)/2.0,
                cx_y4=e43*T*np.exp(-e44*T/2.0)/2.0,
                Dy1=np.exp(-g*T))


def _build(consts, loop_m=None):
    """Build the Bass program. loop_m: if set, wrap the integration steps in a
    hardware For_i loop repeated loop_m times (timing variant only)."""
    import concourse.bacc as bacc
    import concourse.tile as tile
    from concourse import mybir

    F = mybir.ActivationFunctionType
    OP = mybir.AluOpType
    f32 = mybir.dt.float32

    dK = float(consts['dK']); hK = float(consts['hK'])

    nc = bacc.Bacc("TRN2", target_bir_lowering=False, debug=False,
                   num_devices=N_CORES)
    y_in = nc.dram_tensor("y_in", [B_CORE, 6], f32, kind="ExternalInput").ap()
    y_out = nc.dram_tensor("y_out", [B_CORE, 6], f32, kind="ExternalOutput").ap()

    src = y_in.rearrange("(p j) k -> p (j k)", p=P)
    dst = y_out.rearrange("(p j) k -> p (j k)", p=P)

    with tile.TileContext(nc) as tc:
        import contextlib
        with contextlib.ExitStack() as ctx:
            state = ctx.enter_context(tc.tile_pool(name="state", bufs=1))
            temps = ctx.enter_context(tc.tile_pool(name="temps", bufs=2))
            io = ctx.enter_context(tc.tile_pool(name="io", bufs=1))

            in_tile = io.tile([P, JTOT*6], f32, name="in_tile")
            out_tile = io.tile([P, JTOT*6], f32, name="out_tile")
            in3 = in_tile[:, :].rearrange("p (j k) -> p j k", k=6)
            out3 = out_tile[:, :].rearrange("p (j k) -> p j k", k=6)

            for jb in range(NBLK):
                nc.sync.dma_start(
                    out=in_tile[:, jb*JBLK*6:(jb+1)*JBLK*6],
                    in_=src[:, jb*JBLK*6:(jb+1)*JBLK*6])

            blocks = []
            for jb in range(NBLK):
                js = slice(jb*JBLK, (jb+1)*JBLK)
                st = dict(
                    u=state.tile([P, JBLK], f32, tag=f"u_{jb}", name=f"u_{jb}"),
                    x=state.tile([P, JBLK], f32, tag=f"x_{jb}", name=f"x_{jb}"),
                    S=state.tile([P, JBLK], f32, tag=f"S_{jb}", name=f"S_{jb}"),
                    y0c=state.tile([P, JBLK], f32, tag=f"y0c_{jb}", name=f"y0c_{jb}"),
                    y4c=state.tile([P, JBLK], f32, tag=f"y4c_{jb}", name=f"y4c_{jb}"),
                    js=js)
                # extractions on ACT; S on DVE (strided reads are fine)
                nc.scalar.copy(st['u'], in3[:, js, 2])
                nc.scalar.copy(st['x'], in3[:, js, 3])
                nc.scalar.copy(st['y0c'], in3[:, js, 0])
                nc.scalar.copy(st['y4c'], in3[:, js, 4])
                nc.vector.tensor_add(st['S'], in3[:, js, 3], in3[:, js, 5])
                blocks.append(st)

            LN2 = float(np.log(2.0))

            def slow0(st, jb):
                """Initial half slow application (j=0), direct form."""
                u, x, js = st['u'], st['x'], st['js']
                al = float(consts['alphas'][0]); be = float(consts['betas'][0])
                m22 = float(consts['m22s'][0]); m23 = float(consts['m23s'][0])
                srct = temps.tile([P, JBLK], f32, tag=f"srct_{jb}",
                                  name=f"srct0_{jb}")
                nc.vector.tensor_scalar_mul(srct, st['y0c'], al)
                nc.vector.scalar_tensor_tensor(srct, st['y4c'], be, srct,
                                               op0=OP.mult, op1=OP.add)
                nc.vector.scalar_tensor_tensor(u, u, m22, srct,
                                               op0=OP.mult, op1=OP.add)
                nc.vector.scalar_tensor_tensor(u, x, m23, u,
                                               op0=OP.mult, op1=OP.add)

            def full_step(st, j, jb, i, final=False):
                """One fast step followed (algebraically merged) by slow app j.
                u_new = G + (m23-m22)*xi, G = m22*u0 + SRC + m23*x0 (off-chain)."""
                u, x, S = st['u'], st['x'], st['S']
                js = st['js']
                al = float(consts['alphas'][j]); be = float(consts['betas'][j])
                m22 = float(consts['m22s'][j]); m23 = float(consts['m23s'][j])
                t = lambda nm: temps.tile([P, JBLK], f32, tag=f"{nm}_{jb}",
                                          name=f"{nm}_{jb}_{i}")
                # fast chain
                v = t('v');   nc.vector.tensor_sub(v, S, x)
                Bt = t('Bt'); nc.vector.scalar_tensor_tensor(Bt, u, dK, v,
                                                             op0=OP.add, op1=OP.add)
                p = t('p');   nc.vector.tensor_mul(p, u, v)
                n1 = t('n1'); nc.vector.scalar_tensor_tensor(n1, x, -dK, p,
                                                             op0=OP.mult, op1=OP.add)
                Bsq = t('Bsq'); nc.vector.tensor_mul(Bsq, Bt, Bt)
                dsc = t('dsc'); nc.vector.scalar_tensor_tensor(dsc, n1, -4.0, Bsq,
                                                               op0=OP.mult, op1=OP.add)
                l1 = t('l1'); nc.scalar.activation(l1, dsc, F.Ln)
                sd = t('sd'); nc.scalar.activation(sd, l1, F.Exp, scale=0.5)
                # off-chain G emitted here: fills the DVE queue while ACT
                # computes l1/sd (engine queues are in-order)
                G = t('G')
                nc.vector.tensor_scalar_mul(G, st['y0c'], al)
                nc.vector.scalar_tensor_tensor(G, st['y4c'], be, G,
                                               op0=OP.mult, op1=OP.add)
                nc.vector.scalar_tensor_tensor(G, u, m22, G,
                                               op0=OP.mult, op1=OP.add)
                nc.vector.scalar_tensor_tensor(G, x, m23, G,
                                               op0=OP.mult, op1=OP.add)
                bm = t('bm'); nc.vector.tensor_sub(bm, Bt, sd)
                bp = t('bp'); nc.vector.tensor_add(bp, Bt, sd)
                E = t('E');   nc.scalar.activation(E, sd, F.Exp, scale=-hK)
                oE = t('oE'); nc.vector.tensor_scalar(oE, E, -1.0, 1.0,
                                                      op0=OP.mult, op1=OP.add)
                n2 = t('n2'); nc.vector.tensor_mul(n2, n1, oE)
                bmE = t('bmE'); nc.vector.tensor_mul(bmE, bm, E)
                den = t('den'); nc.vector.tensor_sub(den, bp, bmE)
                di = t('di'); nc.vector.reciprocal_approx_fast(di, den)
                xi = t('xi'); nc.vector.scalar_tensor_tensor(xi, n2, 2.0, di,
                                                             op0=OP.mult, op1=OP.mult)
                nc.vector.tensor_add(x, x, xi)
                u_out = out3[:, js, 2] if final else u
                nc.vector.scalar_tensor_tensor(u_out, xi, m23 - m22, G,
                                               op0=OP.mult, op1=OP.add)

            # integration
            for jb, st in enumerate(blocks):
                slow0(st, jb)
            if loop_m is None:
                for i in range(N_STEPS):
                    last = (i == N_STEPS - 1)
                    for jb, st in enumerate(blocks):
                        full_step(st, i+1, jb, i, final=last)
            else:
                with tc.For_i(0, loop_m, 1):
                    for i in range(N_STEPS):
                        for jb, st in enumerate(blocks):
                            full_step(st, 1, jb, i, final=False)

            # final assembly per block
            cx0 = float(consts['cx_y0']); cx4 = float(consts['cx_y4'])
            for jb, st in enumerate(blocks):
                js = st['js']
                x, S = st['x'], st['S']
                x0ap = in3[:, js, 3]
                tl = lambda nm: temps.tile([P, JBLK], f32, tag=f"{nm}_{jb}",
                                           name=f"{nm}_f{jb}")
                # y0_out = Dy0T*y0(0) + cx0*(x0 + xT)
                f0 = tl('f0')
                nc.scalar.activation(f0, in3[:, js, 0], F.Copy,
                                     scale=float(consts['Dy0T']))
                nc.vector.scalar_tensor_tensor(f0, x0ap, cx0, f0,
                                               op0=OP.mult, op1=OP.add)
                nc.vector.scalar_tensor_tensor(out3[:, js, 0], x, cx0, f0,
                                               op0=OP.mult, op1=OP.add)
                # y4_out = Dy4T*y4(0) + PsiT*y0(0) + cx4*(x0 + xT)
                f4 = tl('f4')
                nc.scalar.activation(f4, in3[:, js, 4], F.Copy,
                                     scale=float(consts['Dy4T']))
                nc.vector.scalar_tensor_tensor(f4, in3[:, js, 0],
                                               float(consts['PsiT']), f4,
                                               op0=OP.mult, op1=OP.add)
                nc.vector.scalar_tensor_tensor(f4, x0ap, cx4, f4,
                                               op0=OP.mult, op1=OP.add)
                nc.vector.scalar_tensor_tensor(out3[:, js, 4], x, cx4, f4,
                                               op0=OP.mult, op1=OP.add)
                # y1, x, y5
                nc.scalar.activation(out3[:, js, 1], in3[:, js, 1], F.Copy,
                                     scale=float(consts['Dy1']))
                nc.scalar.copy(out3[:, js, 3], x)
                nc.vector.tensor_sub(out3[:, js, 5], S, x)
                nc.sync.dma_start(
                    out=dst[:, jb*JBLK*6:(jb+1)*JBLK*6],
                    in_=out_tile[:, jb*JBLK*6:(jb+1)*JBLK*6])

    nc.compile()
    return nc


def _get_program(c_arr, T):
    key = (float(T), N_STEPS, NBLK, c_arr.astype(np.float64).tobytes())
    if key not in _cache:
        consts = _derive(c_arr.astype(np.float64), float(T), N_STEPS)
        _cache[key] = _build(consts)
    return _cache[key]


def kernel(y, w, c, t, deltaT):
    y = np.asarray(y, dtype=np.float32)
    w = np.asarray(w, dtype=np.float32)
    c = np.asarray(c, dtype=np.float32)
    t = np.float32(np.asarray(t))
    deltaT = np.float32(np.asarray(deltaT))

    nc = _get_program(c, float(deltaT))

    from concourse.bass_utils import run_bass_kernel_spmd
    shards = [y[i*B_CORE:(i+1)*B_CORE] for i in range(N_CORES)]
    res = run_bass_kernel_spmd(nc, [{"y_in": s} for s in shards],
                               core_ids=list(range(N_CORES)))
    y_new = np.concatenate([res.results[i]["y_out"] for i in range(N_CORES)],
                           axis=0).astype(np.float32)
    return (y_new, w, c, np.float32(t + deltaT))


if __name__ == "__main__":
    rng = np.random.default_rng(0)
    scales = np.array([4.51631477927063, 1e-3, 1e-3, 1e-3, 1e-3, 4.982e-05],
                      dtype=np.float32)
    y = (rng.uniform(size=(B_TOTAL, 6)).astype(np.float32) * scales)
    C_CONST = np.array([0.0, 0.0, 1000.0, 55.0, 45.0, 5.0, 181913.0, 148920.0,
                        364.0, 559000000.0, 23.9, 26.6, 0.55, 0.715, 0.95, 0.2,
                        4.51631477927063, 0.0, 0.0, 0.0, 0.0, 0.0, 0.0,
                        4.982e-05, 1000.0, 55.0, 45.0, 5.0], dtype=np.float32)
    out = kernel(y, np.zeros(0, np.float32), C_CONST, np.float32(0.0),
                 np.float32(0.1))
    print("y_new[:2] =", out[0][:2])
    print("finite:", np.isfinite(out[0]).all())


# revision 15
# speedup vs baseline: 1.0350x; 1.0350x over previous
"""Trainium2 Bass kernel for nn_ModelStep_12120397709837.

Integrates the 6-state SBML pharmacokinetic ODE for B=524288 trajectories over
deltaT, matching jax.experimental.ode.odeint(rtol=atol=1e-6) semantics.

Method (per trajectory, fixed op sequence, no data-dependent control flow):
  - y1 is exact exponential decay (one constant multiply).
  - y3 + y5 = S is exactly conserved; y5 is eliminated.
  - y0 and y4 follow closed-form linear decays; their tiny x-coupling is
    applied once at the end via a trapezoid correction.
  - Strang splitting with N_STEPS uniform steps on (u=y2, x=y3):
      * slow flow: u <- M22*u + M23*x + alpha_j*y0(0) + beta_j*y4(0), with all
        coefficients precomputed on host from the 4x4 matrix exponential;
      * fast flow: reversible bimolecular binding (u + v <-> x), solved
        EXACTLY via the closed-form Riccati solution:
          xi(h) = 2*C*(1-E) / (bp - bm*E),  E = exp(-h*K*sd)
          C = u*v-(d/K)*x, B = u+v+d/K, sd = sqrt(B^2-4C), bm/bp = B-/+sd
  sqrt and reciprocal are computed as Exp(0.5*Ln(.)) / Exp(-Ln(.)) so all ACT
  functions stay in one table set (natural_log_exp_and_others) - no reloads.

Sharding: data-parallel over the batch axis, 65536 trajectories per core on
8 NeuronCores, laid out [128 partitions x 512].
"""
import sys
sys.path.insert(0, '/opt/trn_rl_repo')

import numpy as np

N_CORES = 8
B_TOTAL = 524288
B_CORE = B_TOTAL // N_CORES          # 65536
P = 128                              # SBUF partitions
JTOT = B_CORE // P                   # 512 trajectories per partition
NBLK = 1                             # column blocks
JBLK = JTOT // NBLK
N_STEPS = 8

_cache = {}


def _derive(c64, T, n):
    """Host-side f64 derivation of all scalar constants baked into the program."""
    from scipy.linalg import expm
    c = c64
    a   = (c[6]-c[8] + (1.0-c[14])*c[8] + c[12]) / (c[24]*1000.0)
    b   = c[12]*c[13] / (c[24]*5.0)
    g   = c[7] / (c[25]*55.0)
    c20 = c[12] / (c[27]*1000.0)
    c24 = c[12] / (c[27]*45.0)
    c23 = c[10] / 5.0
    c22 = c[11] / 5.0
    K   = c[9] / 25.0
    d   = c[10]/5.0 + c[12]/(c[27]*5.0)
    e40 = (1.0-c[14])*c[8] / (c[26]*1000.0)
    e43 = c[12]*(1.0-c[13]) / (c[26]*5.0)
    e44 = ((1.0-c[15])*c[8]/45.0 + c[12]/45.0) / c[26]
    h = T / n
    A = np.array([
        [-a,    0.0,  0.0,  b],
        [e40, -e44,   0.0,  e43],
        [c20,  c24, -c22,  (c23 - d)],
        [0.0,  0.0,  0.0,  0.0]])

    def psi(t):  # y0(0) -> y4(t) double-exponential response
        return e40*(np.exp(-a*t)-np.exp(-e44*t))/(e44-a)

    # slow application j: j=0 half step (before first fast), j=1..n-1 full,
    # j=n half (after last fast). Tj = cumulative slow time before app j.
    alphas, betas, m22s, m23s = [], [], [], []
    for j in range(n+1):
        hh = h/2 if j in (0, n) else h
        Tj = 0.0 if j == 0 else h/2 + (j-1)*h
        M = expm(A*hh)
        alphas.append(M[2, 0]*np.exp(-a*Tj) + M[2, 1]*psi(Tj))
        betas.append(M[2, 1]*np.exp(-e44*Tj))
        m22s.append(M[2, 2])
        m23s.append(M[2, 3])
    return dict(alphas=alphas, betas=betas, m22s=m22s, m23s=m23s,
                dK=d/K, hK=h*K,
                Dy0T=np.exp(-a*T), Dy4T=np.exp(-e44*T), PsiT=psi(T),
                cx_y0=b*T*np.exp(-a*T/2.0)/2.0,
                cx_y4=e43*T*np.exp(-e44*T/2.0)/2.0,
                Dy1=np.exp(-g*T))


def _build(consts, loop_m=None):
    """Build the Bass program. loop_m: if set, wrap the integration steps in a
    hardware For_i loop repeated loop_m times (timing variant only)."""
    import concourse.bacc as bacc
    import concourse.tile as tile
    from concourse import mybir

    F = mybir.ActivationFunctionType
    OP = mybir.AluOpType
    f32 = mybir.dt.float32

    dK = float(consts['dK']); hK = float(consts['hK'])

    nc = bacc.Bacc("TRN2", target_bir_lowering=False, debug=False,
                   num_devices=N_CORES)
    y_in = nc.dram_tensor("y_in", [B_CORE, 6], f32, kind="ExternalInput").ap()
    y_out = nc.dram_tensor("y_out", [B_CORE, 6], f32, kind="ExternalOutput").ap()

    src = y_in.rearrange("(p j) k -> p (j k)", p=P)
    dst = y_out.rearrange("(p j) k -> p (j k)", p=P)

    with tile.TileContext(nc) as tc:
        import contextlib
        with contextlib.ExitStack() as ctx:
            state = ctx.enter_context(tc.tile_pool(name="state", bufs=1))
            temps = ctx.enter_context(tc.tile_pool(name="temps", bufs=2))
            io = ctx.enter_context(tc.tile_pool(name="io", bufs=1))

            in_tile = io.tile([P, JTOT*6], f32, name="in_tile")
            out_tile = io.tile([P, JTOT*6], f32, name="out_tile")
            in3 = in_tile[:, :].rearrange("p (j k) -> p j k", k=6)
            out3 = out_tile[:, :].rearrange("p (j k) -> p j k", k=6)

            for jb in range(NBLK):
                nc.sync.dma_start(
                    out=in_tile[:, jb*JBLK*6:(jb+1)*JBLK*6],
                    in_=src[:, jb*JBLK*6:(jb+1)*JBLK*6])

            blocks = []
            for jb in range(NBLK):
                js = slice(jb*JBLK, (jb+1)*JBLK)
                st = dict(
                    u=state.tile([P, JBLK], f32, tag=f"u_{jb}", name=f"u_{jb}"),
                    x=state.tile([P, JBLK], f32, tag=f"x_{jb}", name=f"x_{jb}"),
                    S=state.tile([P, JBLK], f32, tag=f"S_{jb}", name=f"S_{jb}"),
                    y0c=state.tile([P, JBLK], f32, tag=f"y0c_{jb}", name=f"y0c_{jb}"),
                    y4c=state.tile([P, JBLK], f32, tag=f"y4c_{jb}", name=f"y4c_{jb}"),
                    js=js)
                # extractions on ACT; S on DVE (strided reads are fine)
                nc.scalar.copy(st['u'], in3[:, js, 2])
                nc.scalar.copy(st['x'], in3[:, js, 3])
                nc.scalar.copy(st['y0c'], in3[:, js, 0])
                nc.scalar.copy(st['y4c'], in3[:, js, 4])
                nc.vector.tensor_add(st['S'], in3[:, js, 3], in3[:, js, 5])
                blocks.append(st)

            LN2 = float(np.log(2.0))

            def slow0(st, jb):
                """Initial half slow application (j=0), direct form."""
                u, x, js = st['u'], st['x'], st['js']
                al = float(consts['alphas'][0]); be = float(consts['betas'][0])
                m22 = float(consts['m22s'][0]); m23 = float(consts['m23s'][0])
                srct = temps.tile([P, JBLK], f32, tag=f"srct_{jb}",
                                  name=f"srct0_{jb}")
                nc.vector.tensor_scalar_mul(srct, st['y0c'], al)
                nc.vector.scalar_tensor_tensor(srct, st['y4c'], be, srct,
                                               op0=OP.mult, op1=OP.add)
                nc.vector.scalar_tensor_tensor(u, u, m22, srct,
                                               op0=OP.mult, op1=OP.add)
                nc.vector.scalar_tensor_tensor(u, x, m23, u,
                                               op0=OP.mult, op1=OP.add)

            def full_step(st, j, jb, i, final=False):
                """One fast step followed (algebraically merged) by slow app j.
                u_new = G + (m23-m22)*xi, G = m22*u0 + SRC + m23*x0 (off-chain)."""
                u, x, S = st['u'], st['x'], st['S']
                js = st['js']
                al = float(consts['alphas'][j]); be = float(consts['betas'][j])
                m22 = float(consts['m22s'][j]); m23 = float(consts['m23s'][j])
                t = lambda nm: temps.tile([P, JBLK], f32, tag=f"{nm}_{jb}",
                                          name=f"{nm}_{jb}_{i}")
                # fast chain
                v = t('v');   nc.vector.tensor_sub(v, S, x)
                Bt = t('Bt'); nc.vector.scalar_tensor_tensor(Bt, u, dK, v,
                                                             op0=OP.add, op1=OP.add)
                p = t('p');   nc.vector.tensor_mul(p, u, v)
                n1 = t('n1'); nc.vector.scalar_tensor_tensor(n1, x, -dK, p,
                                                             op0=OP.mult, op1=OP.add)
                Bsq = t('Bsq'); nc.scalar.activation(Bsq, Bt, F.Square)
                dsc = t('dsc'); nc.vector.scalar_tensor_tensor(dsc, n1, -4.0, Bsq,
                                                               op0=OP.mult, op1=OP.add)
                l1 = t('l1'); nc.scalar.activation(l1, dsc, F.Ln)
                sd = t('sd'); nc.scalar.activation(sd, l1, F.Exp, scale=0.5)
                # off-chain G emitted here: fills the DVE queue while ACT
                # computes l1/sd (engine queues are in-order)
                G = t('G')
                nc.vector.tensor_scalar_mul(G, st['y0c'], al)
                nc.vector.scalar_tensor_tensor(G, st['y4c'], be, G,
                                               op0=OP.mult, op1=OP.add)
                nc.vector.scalar_tensor_tensor(G, u, m22, G,
                                               op0=OP.mult, op1=OP.add)
                nc.vector.scalar_tensor_tensor(G, x, m23, G,
                                               op0=OP.mult, op1=OP.add)
                bm = t('bm'); nc.vector.tensor_sub(bm, Bt, sd)
                bp = t('bp'); nc.vector.tensor_add(bp, Bt, sd)
                E = t('E');   nc.scalar.activation(E, sd, F.Exp, scale=-hK)
                oE = t('oE'); nc.scalar.activation(oE, E, F.Copy, scale=-1.0, bias=1.0)
                n2 = t('n2'); nc.vector.tensor_mul(n2, n1, oE)
                bmE = t('bmE'); nc.vector.tensor_mul(bmE, bm, E)
                den = t('den'); nc.vector.tensor_sub(den, bp, bmE)
                di = t('di'); nc.vector.reciprocal_approx_fast(di, den)
                xi = t('xi'); nc.vector.scalar_tensor_tensor(xi, n2, 2.0, di,
                                                             op0=OP.mult, op1=OP.mult)
                nc.vector.tensor_add(x, x, xi)
                u_out = out3[:, js, 2] if final else u
                nc.vector.scalar_tensor_tensor(u_out, xi, m23 - m22, G,
                                               op0=OP.mult, op1=OP.add)

            # integration
            for jb, st in enumerate(blocks):
                slow0(st, jb)
            if loop_m is None:
                for i in range(N_STEPS):
                    last = (i == N_STEPS - 1)
                    for jb, st in enumerate(blocks):
                        full_step(st, i+1, jb, i, final=last)
            else:
                with tc.For_i(0, loop_m, 1):
                    for i in range(N_STEPS):
                        for jb, st in enumerate(blocks):
                            full_step(st, 1, jb, i, final=False)

            # final assembly per block
            cx0 = float(consts['cx_y0']); cx4 = float(consts['cx_y4'])
            for jb, st in enumerate(blocks):
                js = st['js']
                x, S = st['x'], st['S']
                x0ap = in3[:, js, 3]
                tl = lambda nm: temps.tile([P, JBLK], f32, tag=f"{nm}_{jb}",
                                           name=f"{nm}_f{jb}")
                # y0_out = Dy0T*y0(0) + cx0*(x0 + xT)
                f0 = tl('f0')
                nc.scalar.activation(f0, in3[:, js, 0], F.Copy,
                                     scale=float(consts['Dy0T']))
                nc.vector.scalar_tensor_tensor(f0, x0ap, cx0, f0,
                                               op0=OP.mult, op1=OP.add)
                nc.vector.scalar_tensor_tensor(out3[:, js, 0], x, cx0, f0,
                                               op0=OP.mult, op1=OP.add)
                # y4_out = Dy4T*y4(0) + PsiT*y0(0) + cx4*(x0 + xT)
                f4 = tl('f4')
                nc.scalar.activation(f4, in3[:, js, 4], F.Copy,
                                     scale=float(consts['Dy4T']))
                nc.vector.scalar_tensor_tensor(f4, in3[:, js, 0],
                                               float(consts['PsiT']), f4,
                                               op0=OP.mult, op1=OP.add)
                nc.vector.scalar_tensor_tensor(f4, x0ap, cx4, f4,
                                               op0=OP.mult, op1=OP.add)
                nc.vector.scalar_tensor_tensor(out3[:, js, 4], x, cx4, f4,
                                               op0=OP.mult, op1=OP.add)
                # y1, x, y5
                nc.scalar.activation(out3[:, js, 1], in3[:, js, 1], F.Copy,
                                     scale=float(consts['Dy1']))
                nc.scalar.copy(out3[:, js, 3], x)
                nc.vector.tensor_sub(out3[:, js, 5], S, x)
                nc.sync.dma_start(
                    out=dst[:, jb*JBLK*6:(jb+1)*JBLK*6],
                    in_=out_tile[:, jb*JBLK*6:(jb+1)*JBLK*6])

    nc.compile()
    return nc


def _get_program(c_arr, T):
    key = (float(T), N_STEPS, NBLK, c_arr.astype(np.float64).tobytes())
    if key not in _cache:
        consts = _derive(c_arr.astype(np.float64), float(T), N_STEPS)
        _cache[key] = _build(consts)
    return _cache[key]


def kernel(y, w, c, t, deltaT):
    y = np.asarray(y, dtype=np.float32)
    w = np.asarray(w, dtype=np.float32)
    c = np.asarray(c, dtype=np.float32)
    t = np.float32(np.asarray(t))
    deltaT = np.float32(np.asarray(deltaT))

    nc = _get_program(c, float(deltaT))

    from concourse.bass_utils import run_bass_kernel_spmd
    shards = [y[i*B_CORE:(i+1)*B_CORE] for i in range(N_CORES)]
    res = run_bass_kernel_spmd(nc, [{"y_in": s} for s in shards],
                               core_ids=list(range(N_CORES)))
    y_new = np.concatenate([res.results[i]["y_out"] for i in range(N_CORES)],
                           axis=0).astype(np.float32)
    return (y_new, w, c, np.float32(t + deltaT))


if __name__ == "__main__":
    rng = np.random.default_rng(0)
    scales = np.array([4.51631477927063, 1e-3, 1e-3, 1e-3, 1e-3, 4.982e-05],
                      dtype=np.float32)
    y = (rng.uniform(size=(B_TOTAL, 6)).astype(np.float32) * scales)
    C_CONST = np.array([0.0, 0.0, 1000.0, 55.0, 45.0, 5.0, 181913.0, 148920.0,
                        364.0, 559000000.0, 23.9, 26.6, 0.55, 0.715, 0.95, 0.2,
                        4.51631477927063, 0.0, 0.0, 0.0, 0.0, 0.0, 0.0,
                        4.982e-05, 1000.0, 55.0, 45.0, 5.0], dtype=np.float32)
    out = kernel(y, np.zeros(0, np.float32), C_CONST, np.float32(0.0),
                 np.float32(0.1))
    print("y_new[:2] =", out[0][:2])
    print("finite:", np.isfinite(out[0]).all())


# revision 16
# speedup vs baseline: 1.0758x; 1.0394x over previous
"""Trainium2 Bass kernel for nn_ModelStep_12120397709837.

Integrates the 6-state SBML pharmacokinetic ODE for B=524288 trajectories over
deltaT, matching jax.experimental.ode.odeint(rtol=atol=1e-6) semantics.

Method (per trajectory, fixed op sequence, no data-dependent control flow):
  - y1 is exact exponential decay (one constant multiply).
  - y3 + y5 = S is exactly conserved; y5 is eliminated.
  - y0 and y4 follow closed-form linear decays; their tiny x-coupling is
    applied once at the end via a trapezoid correction.
  - Strang splitting with N_STEPS uniform steps on (u=y2, x=y3):
      * slow flow: u <- M22*u + M23*x + alpha_j*y0(0) + beta_j*y4(0), with all
        coefficients precomputed on host from the 4x4 matrix exponential;
      * fast flow: reversible bimolecular binding (u + v <-> x), solved
        EXACTLY via the closed-form Riccati solution:
          xi(h) = 2*C*(1-E) / (bp - bm*E),  E = exp(-h*K*sd)
          C = u*v-(d/K)*x, B = u+v+d/K, sd = sqrt(B^2-4C), bm/bp = B-/+sd
  sqrt and reciprocal are computed as Exp(0.5*Ln(.)) / Exp(-Ln(.)) so all ACT
  functions stay in one table set (natural_log_exp_and_others) - no reloads.

Sharding: data-parallel over the batch axis, 65536 trajectories per core on
8 NeuronCores, laid out [128 partitions x 512].
"""
import sys
sys.path.insert(0, '/opt/trn_rl_repo')

import numpy as np

N_CORES = 8
B_TOTAL = 524288
B_CORE = B_TOTAL // N_CORES          # 65536
P = 128                              # SBUF partitions
JTOT = B_CORE // P                   # 512 trajectories per partition
NBLK = 1                             # column blocks
JBLK = JTOT // NBLK
N_STEPS = 8

_cache = {}


def _derive(c64, T, n):
    """Host-side f64 derivation of all scalar constants baked into the program."""
    from scipy.linalg import expm
    c = c64
    a   = (c[6]-c[8] + (1.0-c[14])*c[8] + c[12]) / (c[24]*1000.0)
    b   = c[12]*c[13] / (c[24]*5.0)
    g   = c[7] / (c[25]*55.0)
    c20 = c[12] / (c[27]*1000.0)
    c24 = c[12] / (c[27]*45.0)
    c23 = c[10] / 5.0
    c22 = c[11] / 5.0
    K   = c[9] / 25.0
    d   = c[10]/5.0 + c[12]/(c[27]*5.0)
    e40 = (1.0-c[14])*c[8] / (c[26]*1000.0)
    e43 = c[12]*(1.0-c[13]) / (c[26]*5.0)
    e44 = ((1.0-c[15])*c[8]/45.0 + c[12]/45.0) / c[26]
    h = T / n
    A = np.array([
        [-a,    0.0,  0.0,  b],
        [e40, -e44,   0.0,  e43],
        [c20,  c24, -c22,  (c23 - d)],
        [0.0,  0.0,  0.0,  0.0]])

    def psi(t):  # y0(0) -> y4(t) double-exponential response
        return e40*(np.exp(-a*t)-np.exp(-e44*t))/(e44-a)

    # slow application j: j=0 half step (before first fast), j=1..n-1 full,
    # j=n half (after last fast). Tj = cumulative slow time before app j.
    alphas, betas, m22s, m23s = [], [], [], []
    for j in range(n+1):
        hh = h/2 if j in (0, n) else h
        Tj = 0.0 if j == 0 else h/2 + (j-1)*h
        M = expm(A*hh)
        alphas.append(M[2, 0]*np.exp(-a*Tj) + M[2, 1]*psi(Tj))
        betas.append(M[2, 1]*np.exp(-e44*Tj))
        m22s.append(M[2, 2])
        m23s.append(M[2, 3])
    return dict(alphas=alphas, betas=betas, m22s=m22s, m23s=m23s,
                dK=d/K, hK=h*K,
                Dy0T=np.exp(-a*T), Dy4T=np.exp(-e44*T), PsiT=psi(T),
                cx_y0=b*T*np.exp(-a*T/2.0)/2.0,
                cx_y4=e43*T*np.exp(-e44*T/2.0)/2.0,
                Dy1=np.exp(-g*T))


def _build(consts, loop_m=None):
    """Build the Bass program. loop_m: if set, wrap the integration steps in a
    hardware For_i loop repeated loop_m times (timing variant only)."""
    import concourse.bacc as bacc
    import concourse.tile as tile
    from concourse import mybir

    F = mybir.ActivationFunctionType
    OP = mybir.AluOpType
    f32 = mybir.dt.float32

    dK = float(consts['dK']); hK = float(consts['hK'])

    nc = bacc.Bacc("TRN2", target_bir_lowering=False, debug=False,
                   num_devices=N_CORES)
    y_in = nc.dram_tensor("y_in", [B_CORE, 6], f32, kind="ExternalInput").ap()
    y_out = nc.dram_tensor("y_out", [B_CORE, 6], f32, kind="ExternalOutput").ap()

    src = y_in.rearrange("(p j) k -> p (j k)", p=P)
    dst = y_out.rearrange("(p j) k -> p (j k)", p=P)

    with tile.TileContext(nc) as tc:
        import contextlib
        with contextlib.ExitStack() as ctx:
            state = ctx.enter_context(tc.tile_pool(name="state", bufs=1))
            temps = ctx.enter_context(tc.tile_pool(name="temps", bufs=2))
            io = ctx.enter_context(tc.tile_pool(name="io", bufs=1))

            in_tile = io.tile([P, JTOT*6], f32, name="in_tile")
            out_tile = io.tile([P, JTOT*6], f32, name="out_tile")
            in3 = in_tile[:, :].rearrange("p (j k) -> p j k", k=6)
            out3 = out_tile[:, :].rearrange("p (j k) -> p j k", k=6)

            # Pre-warm the natural_log_exp_and_others ACT table set while the
            # input DMA streams (hides the ~2.7us table load). The dummy Ln
            # output lands in out_tile[0,0] and is overwritten by the real
            # y0 write during final assembly.
            warm = io.tile([P, 1], f32, name="warm")
            nc.vector.memset(warm, 1.0)
            nc.scalar.activation(out3[:, 0:1, 0], warm, F.Ln)

            for jb in range(NBLK):
                nc.sync.dma_start(
                    out=in_tile[:, jb*JBLK*6:(jb+1)*JBLK*6],
                    in_=src[:, jb*JBLK*6:(jb+1)*JBLK*6])

            blocks = []
            for jb in range(NBLK):
                js = slice(jb*JBLK, (jb+1)*JBLK)
                st = dict(
                    u=state.tile([P, JBLK], f32, tag=f"u_{jb}", name=f"u_{jb}"),
                    x=state.tile([P, JBLK], f32, tag=f"x_{jb}", name=f"x_{jb}"),
                    S=state.tile([P, JBLK], f32, tag=f"S_{jb}", name=f"S_{jb}"),
                    y0c=state.tile([P, JBLK], f32, tag=f"y0c_{jb}", name=f"y0c_{jb}"),
                    y4c=state.tile([P, JBLK], f32, tag=f"y4c_{jb}", name=f"y4c_{jb}"),
                    js=js)
                # extractions on ACT; S on DVE (strided reads are fine)
                nc.scalar.copy(st['u'], in3[:, js, 2])
                nc.scalar.copy(st['x'], in3[:, js, 3])
                nc.scalar.copy(st['y0c'], in3[:, js, 0])
                nc.scalar.copy(st['y4c'], in3[:, js, 4])
                nc.vector.tensor_add(st['S'], in3[:, js, 3], in3[:, js, 5])
                blocks.append(st)

            LN2 = float(np.log(2.0))

            def slow0(st, jb):
                """Initial half slow application (j=0), direct form."""
                u, x, js = st['u'], st['x'], st['js']
                al = float(consts['alphas'][0]); be = float(consts['betas'][0])
                m22 = float(consts['m22s'][0]); m23 = float(consts['m23s'][0])
                srct = temps.tile([P, JBLK], f32, tag=f"srct_{jb}",
                                  name=f"srct0_{jb}")
                nc.vector.tensor_scalar_mul(srct, st['y0c'], al)
                nc.vector.scalar_tensor_tensor(srct, st['y4c'], be, srct,
                                               op0=OP.mult, op1=OP.add)
                nc.vector.scalar_tensor_tensor(u, u, m22, srct,
                                               op0=OP.mult, op1=OP.add)
                nc.vector.scalar_tensor_tensor(u, x, m23, u,
                                               op0=OP.mult, op1=OP.add)

            def full_step(st, j, jb, i, final=False):
                """One fast step followed (algebraically merged) by slow app j.
                u_new = G + (m23-m22)*xi, G = m22*u0 + SRC + m23*x0 (off-chain)."""
                u, x, S = st['u'], st['x'], st['S']
                js = st['js']
                al = float(consts['alphas'][j]); be = float(consts['betas'][j])
                m22 = float(consts['m22s'][j]); m23 = float(consts['m23s'][j])
                t = lambda nm: temps.tile([P, JBLK], f32, tag=f"{nm}_{jb}",
                                          name=f"{nm}_{jb}_{i}")
                # fast chain
                v = t('v');   nc.vector.tensor_sub(v, S, x)
                Bt = t('Bt'); nc.vector.scalar_tensor_tensor(Bt, u, dK, v,
                                                             op0=OP.add, op1=OP.add)
                p = t('p');   nc.vector.tensor_mul(p, u, v)
                n1 = t('n1'); nc.vector.scalar_tensor_tensor(n1, x, -dK, p,
                                                             op0=OP.mult, op1=OP.add)
                Bsq = t('Bsq'); nc.scalar.activation(Bsq, Bt, F.Square)
                dsc = t('dsc'); nc.vector.scalar_tensor_tensor(dsc, n1, -4.0, Bsq,
                                                               op0=OP.mult, op1=OP.add)
                l1 = t('l1'); nc.scalar.activation(l1, dsc, F.Ln)
                sd = t('sd'); nc.scalar.activation(sd, l1, F.Exp, scale=0.5)
                # off-chain G emitted here: fills the DVE queue while ACT
                # computes l1/sd (engine queues are in-order)
                G = t('G')
                nc.vector.tensor_scalar_mul(G, st['y0c'], al)
                nc.vector.scalar_tensor_tensor(G, st['y4c'], be, G,
                                               op0=OP.mult, op1=OP.add)
                nc.vector.scalar_tensor_tensor(G, u, m22, G,
                                               op0=OP.mult, op1=OP.add)
                nc.vector.scalar_tensor_tensor(G, x, m23, G,
                                               op0=OP.mult, op1=OP.add)
                bm = t('bm'); nc.vector.tensor_sub(bm, Bt, sd)
                bp = t('bp'); nc.vector.tensor_add(bp, Bt, sd)
                E = t('E');   nc.scalar.activation(E, sd, F.Exp, scale=-hK)
                oE = t('oE'); nc.scalar.activation(oE, E, F.Copy, scale=-1.0, bias=1.0)
                n2 = t('n2'); nc.vector.tensor_mul(n2, n1, oE)
                bmE = t('bmE'); nc.vector.tensor_mul(bmE, bm, E)
                den = t('den'); nc.vector.tensor_sub(den, bp, bmE)
                di = t('di'); nc.vector.reciprocal_approx_fast(di, den)
                xi = t('xi'); nc.vector.scalar_tensor_tensor(xi, n2, 2.0, di,
                                                             op0=OP.mult, op1=OP.mult)
                nc.vector.tensor_add(x, x, xi)
                u_out = out3[:, js, 2] if final else u
                nc.vector.scalar_tensor_tensor(u_out, xi, m23 - m22, G,
                                               op0=OP.mult, op1=OP.add)

            # integration
            for jb, st in enumerate(blocks):
                slow0(st, jb)
            if loop_m is None:
                for i in range(N_STEPS):
                    last = (i == N_STEPS - 1)
                    for jb, st in enumerate(blocks):
                        full_step(st, i+1, jb, i, final=last)
            else:
                with tc.For_i(0, loop_m, 1):
                    for i in range(N_STEPS):
                        for jb, st in enumerate(blocks):
                            full_step(st, 1, jb, i, final=False)

            # final assembly per block
            cx0 = float(consts['cx_y0']); cx4 = float(consts['cx_y4'])
            for jb, st in enumerate(blocks):
                js = st['js']
                x, S = st['x'], st['S']
                x0ap = in3[:, js, 3]
                tl = lambda nm: temps.tile([P, JBLK], f32, tag=f"{nm}_{jb}",
                                           name=f"{nm}_f{jb}")
                # y0_out = Dy0T*y0(0) + cx0*(x0 + xT)
                f0 = tl('f0')
                nc.scalar.activation(f0, in3[:, js, 0], F.Copy,
                                     scale=float(consts['Dy0T']))
                nc.vector.scalar_tensor_tensor(f0, x0ap, cx0, f0,
                                               op0=OP.mult, op1=OP.add)
                nc.vector.scalar_tensor_tensor(out3[:, js, 0], x, cx0, f0,
                                               op0=OP.mult, op1=OP.add)
                # y4_out = Dy4T*y4(0) + PsiT*y0(0) + cx4*(x0 + xT)
                f4 = tl('f4')
                nc.scalar.activation(f4, in3[:, js, 4], F.Copy,
                                     scale=float(consts['Dy4T']))
                nc.vector.scalar_tensor_tensor(f4, in3[:, js, 0],
                                               float(consts['PsiT']), f4,
                                               op0=OP.mult, op1=OP.add)
                nc.vector.scalar_tensor_tensor(f4, x0ap, cx4, f4,
                                               op0=OP.mult, op1=OP.add)
                nc.vector.scalar_tensor_tensor(out3[:, js, 4], x, cx4, f4,
                                               op0=OP.mult, op1=OP.add)
                # y1, x, y5
                nc.scalar.activation(out3[:, js, 1], in3[:, js, 1], F.Copy,
                                     scale=float(consts['Dy1']))
                nc.scalar.copy(out3[:, js, 3], x)
                nc.vector.tensor_sub(out3[:, js, 5], S, x)
                nc.sync.dma_start(
                    out=dst[:, jb*JBLK*6:(jb+1)*JBLK*6],
                    in_=out_tile[:, jb*JBLK*6:(jb+1)*JBLK*6])

    nc.compile()
    return nc


def _get_program(c_arr, T):
    key = (float(T), N_STEPS, NBLK, c_arr.astype(np.float64).tobytes())
    if key not in _cache:
        consts = _derive(c_arr.astype(np.float64), float(T), N_STEPS)
        _cache[key] = _build(consts)
    return _cache[key]


def kernel(y, w, c, t, deltaT):
    y = np.asarray(y, dtype=np.float32)
    w = np.asarray(w, dtype=np.float32)
    c = np.asarray(c, dtype=np.float32)
    t = np.float32(np.asarray(t))
    deltaT = np.float32(np.asarray(deltaT))

    nc = _get_program(c, float(deltaT))

    from concourse.bass_utils import run_bass_kernel_spmd
    shards = [y[i*B_CORE:(i+1)*B_CORE] for i in range(N_CORES)]
    res = run_bass_kernel_spmd(nc, [{"y_in": s} for s in shards],
                               core_ids=list(range(N_CORES)))
    y_new = np.concatenate([res.results[i]["y_out"] for i in range(N_CORES)],
                           axis=0).astype(np.float32)
    return (y_new, w, c, np.float32(t + deltaT))


if __name__ == "__main__":
    rng = np.random.default_rng(0)
    scales = np.array([4.51631477927063, 1e-3, 1e-3, 1e-3, 1e-3, 4.982e-05],
                      dtype=np.float32)
    y = (rng.uniform(size=(B_TOTAL, 6)).astype(np.float32) * scales)
    C_CONST = np.array([0.0, 0.0, 1000.0, 55.0, 45.0, 5.0, 181913.0, 148920.0,
                        364.0, 559000000.0, 23.9, 26.6, 0.55, 0.715, 0.95, 0.2,
                        4.51631477927063, 0.0, 0.0, 0.0, 0.0, 0.0, 0.0,
                        4.982e-05, 1000.0, 55.0, 45.0, 5.0], dtype=np.float32)
    out = kernel(y, np.zeros(0, np.float32), C_CONST, np.float32(0.0),
                 np.float32(0.1))
    print("y_new[:2] =", out[0][:2])
    print("finite:", np.isfinite(out[0]).all())
